# revision 45
# baseline (speedup 1.0000x reference)
"""Multi-head attention (B=2, S=2048, E=1024, H=16) on 8 Trainium2 cores.

Sharding: core c -> (batch b = c//4, head-group g = c%4 of 4 heads).
Each core computes Q/K/V projections for its 4 heads (256 features),
full attention for those heads, and a partial output projection
(256 rows of Wo). Host sums the 4 partials per batch element and adds bo.

Schedule (final): built for engine overlap around three facts measured
from NTFF traces: the in-order PE queue executes exactly in emission
order, the exp stream on the Scalar engine is the steady-state floor
(~1.09us per key-tile unit), and the PE p-state only reaches 2.4GHz
when kept continuously busy.
  - x DMAs ride TWO queues (sync+scalar, alternating feature tiles) in
    priority order xq, xv, xk; weights ride the gpsimd queue. y output
    rides sync/gpsimd (NOT scalar: the Scalar engine is the exp
    bottleneck and DMA triggers would steal ~0.7us each from it).
  - Lead-in: Q-proj tracks xq arrival f-inner on 8 psum accumulators,
    then V-proj (token tiles 0-13) tracks xv. K-proj and the last two V
    tiles run as chunks interleaved INTO the attention stream (racing
    the score consumption) so nothing serializes behind the
    last-arriving tensor.
  - Attention is a flat software-pipelined unit stream (qb-major,
    3-deep PV skew): the PE emits scores(u_i) then PV(u_{i-3}), so exp
    latency never blocks the in-order PE queue — including across
    block boundaries.
  - Each block's normalization chain is DEFERRED into the next block's
    kt==3 slot; the output projection for a token range fires right
    after its pair-1 normalization, spreading uniformly over the phase.
  - The last block's output projection drains through the freed score
    psum slots, split across the Scalar and Vector engines.

On-chip layouts (per core):
  qt/kt: (128 feat-part, pair, 2048 tok)  transposed proj outputs; the
         128 partitions hold two heads (64+64) per pair index.
  v:     (128 tok-part, 16 tok-tiles, 4*65): per head 64 dims plus a
         "ones" column produced by an augmented V projection (extra
         output feature with zero weights and bias 1.0); P @ V_aug then
         also yields the softmax denominator row for free.
  scores are computed transposed (key-pos on partitions, query on free)
  so exp runs on ACT along the free dim and P tiles feed P@V directly as
  the moving operand; no transposes anywhere in the pipeline.
"""

import numpy as np

B, S, E, H = 2, 2048, 1024, 16
D = 64
NCORES = 8
FPC = 256  # features (head dims) per core = 4 heads
VW = 4 * 65  # V-projection output width incl. ones columns

MODE = "bf16"

_PROGRAMS = {}
LAST_RESULT = None
TRACE = False
TRACE_DIR = None


def _build(mode):
    import concourse.tile as tile
    from concourse import bacc, mybir

    f32 = mybir.dt.float32
    DT = mybir.dt.bfloat16
    NW = 512
    NNB = S // NW  # 4 query blocks per pair

    nc = bacc.Bacc("TRN2", target_bir_lowering=False, debug=False,
                   num_devices=NCORES)

    xq_ap = nc.dram_tensor("xq", [E, S], DT, kind="ExternalInput").ap()
    xk_ap = nc.dram_tensor("xk", [E, S], DT, kind="ExternalInput").ap()
    xv_ap = nc.dram_tensor("xv", [E, S], DT, kind="ExternalInput").ap()
    wq_ap = nc.dram_tensor("wq", [128, 8, FPC], DT, kind="ExternalInput").ap()
    wk_ap = nc.dram_tensor("wk", [128, 8, FPC], DT, kind="ExternalInput").ap()
    wv_ap = nc.dram_tensor("wv", [128, 8, VW], DT, kind="ExternalInput").ap()
    wo_ap = nc.dram_tensor("wo", [128, 2, E], DT, kind="ExternalInput").ap()
    bqk_ap = nc.dram_tensor("bqk", [128, 4], f32, kind="ExternalInput").ap()
    bv_ap = nc.dram_tensor("bv", [1, VW], DT, kind="ExternalInput").ap()
    ones_ap = nc.dram_tensor("ones", [1, 128], DT, kind="ExternalInput").ap()
    y_ap = nc.dram_tensor("y", [S, E], f32, kind="ExternalOutput").ap()

    Exp = mybir.ActivationFunctionType.Exp

    with tile.TileContext(nc) as tc:
        with tc.tile_pool(name="persist", bufs=1) as persist:
            wq_sb = persist.tile([128, 8, FPC], DT, name="wq_sb")
            wk_sb = persist.tile([128, 8, FPC], DT, name="wk_sb")
            wv_sb = persist.tile([128, 8, VW], DT, name="wv_sb")
            wo_sb = persist.tile([128, 2, E], DT, name="wo_sb")
            bqk_sb = persist.tile([128, 4], f32, name="bqk_sb")
            bv_sb = persist.tile([1, VW], DT, name="bv_sb")
            ones_sb = persist.tile([1, 128], DT, name="ones_sb")
            qt_sb = persist.tile([128, 2, S], DT, name="qt_sb")
            kt_sb = persist.tile([128, 2, S], DT, name="kt_sb")
            v_sb = persist.tile([128, 16, VW], DT, name="v_sb")
            at_sb = persist.tile([128, 2, S], DT, name="at_sb")
            xq_res = persist.tile([128, 8, S], DT, name="xq_res")
            xk_res = persist.tile([128, 8, S], DT, name="xk_res")
            xv_res = persist.tile([128, 8, S], DT, name="xv_res")
            ones32 = ones_sb[:, 0:64]

            # weights/consts on the gpsimd DGE queue, q first (needed first)
            nc.gpsimd.dma_start(wq_sb, wq_ap)
            nc.gpsimd.dma_start(bqk_sb, bqk_ap)
            nc.gpsimd.dma_start(wk_sb, wk_ap)
            nc.gpsimd.dma_start(wv_sb, wv_ap)
            nc.gpsimd.dma_start(bv_sb, bv_ap)
            nc.gpsimd.dma_start(ones_sb, ones_ap)
            nc.gpsimd.dma_start(wo_sb, wo_ap)
            # x on FOUR queues, striped by feature tile, priority
            # xq > xv > xk: attention needs qt+kt+v ready; K-proj is the
            # cheapest PE phase so it tracks the LAST-arriving tensor,
            # while the ldweights-heavy V-projection overlaps xk's DMA
            # window. Four queues issue descriptors in parallel, pushing
            # the aggregate closer to the HBM share.
            xengs = (nc.sync, nc.scalar, nc.gpsimd)
            for xres, xap in ((xq_res, xq_ap), (xv_res, xv_ap),
                              (xk_res, xk_ap)):
                for f in range(8):
                    xengs[f % 3].dma_start(xres[:, f, :],
                                           xap[f * 128:(f + 1) * 128, :])

            def emit_qk_proj(pool, w_sb, x_res, out_sb, bcol):
                """f-inner projection, both pairs: 8 psum accumulators
                track the x feature tiles as they arrive."""
                pj = {}
                for p in range(2):
                    for nb in range(NNB):
                        pj[(p, nb)] = pool.tile(
                            [128, NW], f32, tag="proj", bufs=8,
                            name=f"pj_{bcol}_{p}_{nb}")
                for f in range(8):
                    for p in range(2):
                        for nb in range(NNB):
                            nc.tensor.matmul(
                                pj[(p, nb)],
                                w_sb[:, f, p * 128:(p + 1) * 128],
                                x_res[:, f, nb * NW:(nb + 1) * NW],
                                start=(f == 0), stop=(f == 7))
                for p in range(2):
                    for nb in range(NNB):
                        nc.vector.tensor_scalar_add(
                            out_sb[:, p, nb * NW:(nb + 1) * NW], pj[(p, nb)],
                            bqk_sb[:, bcol + p:bcol + p + 1])

            def emit_v_half(pool, half, ntiles=8):
                """V projection for 8 token tiles, f-inner (tracks xv)."""
                psv = [pool.tile([128, VW], f32, tag="proj", bufs=8,
                                 name=f"pjv_{half}_{i}")
                       for i in range(ntiles)]
                for i in range(ntiles):
                    nc.tensor.matmul(psv[i], ones_sb, bv_sb,
                                     start=True, stop=False)
                for f in range(8):
                    for i in range(ntiles):
                        tt = half * 8 + i
                        nc.tensor.matmul(
                            psv[i],
                            xv_res[:, f, tt * 128:(tt + 1) * 128],
                            wv_sb[:, f, :],
                            start=False, stop=(f == 7))
                for i in range(ntiles):
                    nc.vector.tensor_copy(v_sb[:, half * 8 + i, :], psv[i])

            # ---- lead-in: Q-proj then V-proj (token tiles 0..13) ----
            # K-proj is NOT here: it would serialize behind the V passes
            # in the in-order PE queue. It runs as per-key-block chunks
            # interleaved into the attention stream below, racing the
            # score consumption (xk is resident by then). The last two V
            # tiles also move into the stream so the first scores start
            # sooner.
            with tc.tile_pool(name="lead", bufs=1, space="PSUM") as lead:
                emit_qk_proj(lead, wq_sb, xq_res, qt_sb, 0)
                emit_v_half(lead, 0)
                emit_v_half(lead, 1, ntiles=6)

            # ---- attention: flat unit stream, 3-deep PV skew ----
            # Units are (qb, p, kt); the PE emits scores(u_i) then
            # PV(u_{i-3}) so exp latency on the Scalar engine never blocks
            # the in-order PE queue — including across block boundaries.
            # Each block's normalization chain is deferred into the next
            # block's kt==3 slot (just before that block's pv(0), whose
            # pvt slot rotation waits on this at-mul).
            # psum pool order matters: banks are assigned in order and the
            # released lead-pool's banks free progressively (v-pass2 drains
            # tt8..tt15); misc (first K-proj chunk) and the score slots
            # want the earliest-freed banks.
            with tc.tile_pool(name="pt", bufs=14) as ptpool, \
                 tc.tile_pool(name="sm", bufs=2) as smpool, \
                 tc.tile_pool(name="ysb", bufs=2) as ypool, \
                 tc.tile_pool(name="miscps", bufs=1, space="PSUM") as miscps, \
                 tc.tile_pool(name="scps", bufs=2, space="PSUM") as scps, \
                 tc.tile_pool(name="pvps", bufs=3, space="PSUM") as pvps:

                pvt_of = {}  # (qb, p) -> [h0_tile, h1_tile]

                def emit_yproj_block(qb, last):
                    for mt in range(4 * qb, 4 * qb + 4):
                        yo = ypool.tile([128, E], f32, tag="yo",
                                        name=f"yo_{mt}")
                        for nb in range(2):
                            # the last block's yp tiles use the score psum
                            # slots (free once the exps are done) so the
                            # tail pipelines on 2 slots instead of 1
                            yp = (scps if last else miscps).tile(
                                [128, NW], f32,
                                tag="sc" if last else "misc",
                                name=f"yp_{mt}_{nb}")
                            for p2 in range(2):
                                nc.tensor.matmul(
                                    yp,
                                    at_sb[:, p2, mt * 128:(mt + 1) * 128],
                                    wo_sb[:, p2, nb * NW:(nb + 1) * NW],
                                    start=(p2 == 0), stop=(p2 == 1))
                            # gpsimd cannot read PSUM; drain via DVE, and
                            # split across the (by then idle) Scalar engine
                            # at the tail so the two streams drain in
                            # parallel
                            if last and (mt + nb) % 2 == 0:
                                nc.scalar.copy(
                                    yo[:, nb * NW:(nb + 1) * NW], yp)
                            else:
                                nc.vector.tensor_copy(
                                    yo[:, nb * NW:(nb + 1) * NW], yp)
                        eng = nc.gpsimd if mt % 2 else nc.sync
                        eng.dma_start(y_ap[mt * 128:(mt + 1) * 128, :], yo)

                def emit_norm(qb, p, last=False):
                    qsl = slice(qb * NW, (qb + 1) * NW)
                    pvt = pvt_of.pop((qb, p))
                    for hh in range(2):
                        denr = smpool.tile([1, NW], DT, tag="denr",
                                           name=f"dn_{p}_{qb}_{hh}")
                        nc.vector.tensor_copy(denr, pvt[hh][64:65, :])
                        rb = miscps.tile([64, NW], f32, tag="misc",
                                         name=f"rb_{p}_{qb}_{hh}")
                        nc.tensor.matmul(rb, ones32, denr,
                                         start=True, stop=True)
                        rbs = smpool.tile([64, NW], f32, tag="rbs",
                                          name=f"rbs_{p}_{qb}_{hh}")
                        nc.vector.reciprocal_approx_fast(rbs, rb)
                        nc.vector.tensor_mul(
                            at_sb[64 * hh:64 * hh + 64, p, qsl],
                            pvt[hh][0:64, :], rbs)
                    if p == 1:
                        emit_yproj_block(qb, last)

                def emit_kt_chunk(p, nb):
                    """K projection for one key block (8 f-matmuls on a
                    rotating misc psum slot, DVE bias-add drain)."""
                    pj = miscps.tile([128, NW], f32, tag="misc",
                                     name=f"pk_{p}_{nb}")
                    for f in range(8):
                        nc.tensor.matmul(
                            pj,
                            wk_sb[:, f, p * 128:(p + 1) * 128],
                            xk_res[:, f, nb * NW:(nb + 1) * NW],
                            start=(f == 0), stop=(f == 7))
                    nc.vector.tensor_scalar_add(
                        kt_sb[:, p, nb * NW:(nb + 1) * NW], pj,
                        bqk_sb[:, 2 + p:3 + p])

                def emit_v_chunk(tt):
                    """V projection for one token tile on the misc slot."""
                    psv = miscps.tile([128, VW], f32, tag="misc",
                                      name=f"pjv_s_{tt}")
                    nc.tensor.matmul(psv, ones_sb, bv_sb,
                                     start=True, stop=False)
                    for f in range(8):
                        nc.tensor.matmul(
                            psv,
                            xv_res[:, f, tt * 128:(tt + 1) * 128],
                            wv_sb[:, f, :],
                            start=False, stop=(f == 7))
                    nc.vector.tensor_copy(v_sb[:, tt, :], psv)

                units = [(qb, p, kt)
                         for qb in range(NNB) for p in range(2)
                         for kt in range(16)]
                pend = []  # pv emissions in flight (skew 3)
                # K-proj chunk (p, nb) produces key tiles nb*4..nb*4+3 of
                # pair p; chunk i must precede the first unit consuming it
                # (unit kt==4i%16 of pair i//4 in qb0). Emit chunk 0 up
                # front, then one chunk every 4 units.
                emit_kt_chunk(0, 0)
                kt_chunks = [(p, nb) for p in range(2) for nb in range(4)][1:]

                def emit_pv(qb, p, kt, ptt):
                    if kt == 0:
                        pvt_of[(qb, p)] = [
                            pvps.tile([65, NW], f32, tag="pv",
                                      name=f"pv_{p}_{qb}_{hh}")
                            for hh in range(2)]
                    pvt = pvt_of[(qb, p)]
                    for hh in range(2):
                        h = 2 * p + hh
                        nc.tensor.matmul(
                            pvt[hh],
                            v_sb[:, kt, 65 * h:65 * h + 65],
                            ptt[:, NW * hh:NW * hh + NW],
                            start=(kt == 0), stop=(kt == 15))

                for i, (qb, p, kt) in enumerate(units):
                    if i in (1, 2):
                        emit_v_chunk(13 + i)  # V token tiles 14, 15
                    if kt_chunks and i % 4 == 3:
                        emit_kt_chunk(*kt_chunks.pop(0))
                    qsl = slice(qb * NW, (qb + 1) * NW)
                    s_ = scps.tile([128, 2 * NW], f32, tag="sc",
                                   name=f"sc_{p}_{qb}_{kt}")
                    for hh in range(2):
                        nc.tensor.matmul(
                            s_[:, NW * hh:NW * hh + NW],
                            kt_sb[64 * hh:64 * hh + 64, p,
                                  kt * 128:(kt + 1) * 128],
                            qt_sb[64 * hh:64 * hh + 64, p, qsl],
                            start=True, stop=True)
                    ptt = ptpool.tile([128, 2 * NW], DT, tag="pt",
                                      name=f"pt_{p}_{qb}_{kt}")
                    nc.scalar.activation(ptt, s_, Exp, scale=0.125)
                    pend.append((qb, p, kt, ptt))
                    if kt == 3 and (qb, p) != (0, 0):
                        # previous block's pv(15) was emitted one unit ago;
                        # normalize it now, BEFORE this block's pv(0) below
                        # (whose pvt slot rotation waits on this at-mul).
                        pqb, pp = (qb, p - 1) if p == 1 else (qb - 1, 1)
                        emit_norm(pqb, pp)
                    if len(pend) > 3:
                        emit_pv(*pend.pop(0))
                while pend:
                    emit_pv(*pend.pop(0))
                emit_norm(NNB - 1, 1, last=True)

    nc.compile()
    return nc


def _get_program(mode):
    if mode not in _PROGRAMS:
        _PROGRAMS[mode] = _build(mode)
    return _PROGRAMS[mode]


def kernel(q, k, v, mask, Wq, bq, Wk, bk, Wv, bv, Wo, bo):
    global LAST_RESULT
    from concourse.bass_utils import run_bass_kernel_spmd

    mode = MODE
    nc = _get_program(mode)

    import ml_dtypes
    cdt = ml_dtypes.bfloat16

    def prep(a):
        return np.ascontiguousarray(np.asarray(a).astype(cdt))

    q = np.asarray(q); k = np.asarray(k); v = np.asarray(v)
    Wq = np.asarray(Wq); Wk = np.asarray(Wk); Wv = np.asarray(Wv)
    Wo = np.asarray(Wo)
    bq = np.asarray(bq); bk = np.asarray(bk); bv = np.asarray(bv)
    bo = np.asarray(bo)

    xT = [[prep(q[b].T), prep(k[b].T), prep(v[b].T)] for b in range(B)]

    in_maps = []
    for core in range(NCORES):
        b, g = core // 4, core % 4
        r0 = g * FPC

        def wqk_layout(W):
            # lhsT tiles: [part p, ktile, m] = W.T[kt*128+p, m]
            A = W[r0:r0 + FPC, :].T.reshape(8, 128, FPC)
            return prep(A.transpose(1, 0, 2))

        WvT = Wv[r0:r0 + FPC, :].T  # (E, 256)
        Wv_aug = np.zeros((E, VW), np.float32)
        bv_aug = np.zeros((1, VW), np.float32)
        for h in range(4):
            Wv_aug[:, 65 * h:65 * h + 64] = WvT[:, 64 * h:64 * h + 64]
            bv_aug[0, 65 * h:65 * h + 64] = bv[r0 + 64 * h:r0 + 64 * h + 64]
            bv_aug[0, 65 * h + 64] = 1.0
        Wo_l = Wo[:, r0:r0 + FPC].T.reshape(2, 128, E).transpose(1, 0, 2)

        in_maps.append({
            "xq": xT[b][0], "xk": xT[b][1], "xv": xT[b][2],
            "wq": wqk_layout(Wq),
            "wk": wqk_layout(Wk),
            "wv": prep(Wv_aug.reshape(8, 128, VW).transpose(1, 0, 2)),
            "wo": prep(Wo_l),
            "bqk": np.stack([bq[r0:r0 + 128], bq[r0 + 128:r0 + FPC],
                             bk[r0:r0 + 128], bk[r0 + 128:r0 + FPC]],
                            axis=1).astype(np.float32),
            "bv": prep(bv_aug),
            "ones": np.ones((1, 128), cdt),
        })

    kwargs = {}
    if TRACE:
        kwargs = {"trace": True, "tmpdir": TRACE_DIR}
    res = run_bass_kernel_spmd(nc, in_maps, list(range(NCORES)), **kwargs)
    LAST_RESULT = res

    y = np.zeros((B, S, E), np.float32)
    for core in range(NCORES):
        y[core // 4] += res.results[core]["y"]
    y += bo.astype(np.float32)
    return y


# revision 47
# speedup vs baseline: 1.0456x; 1.0456x over previous
"""Multi-head attention (B=2, S=2048, E=1024, H=16) on 8 Trainium2 cores.

Sharding: core c -> (batch b = c//4, head-group g = c%4 of 4 heads).
Each core computes Q/K/V projections for its 4 heads (256 features),
full attention for those heads, and a partial output projection
(256 rows of Wo). Host sums the 4 partials per batch element and adds bo.

Schedule (final): built for engine overlap around three facts measured
from NTFF traces: the in-order PE queue executes exactly in emission
order, the exp stream on the Scalar engine is the steady-state floor
(~1.09us per key-tile unit), and the PE p-state only reaches 2.4GHz
when kept continuously busy.
  - x DMAs ride TWO queues (sync+scalar, alternating feature tiles) in
    priority order xq, xv, xk; weights ride the gpsimd queue. y output
    rides sync/gpsimd (NOT scalar: the Scalar engine is the exp
    bottleneck and DMA triggers would steal ~0.7us each from it).
  - Lead-in: Q-proj tracks xq arrival f-inner on 8 psum accumulators,
    then V-proj (token tiles 0-13) tracks xv. K-proj and the last two V
    tiles run as chunks interleaved INTO the attention stream (racing
    the score consumption) so nothing serializes behind the
    last-arriving tensor.
  - Attention is a flat software-pipelined unit stream (qb-major,
    3-deep PV skew): the PE emits scores(u_i) then PV(u_{i-3}), so exp
    latency never blocks the in-order PE queue — including across
    block boundaries.
  - Each block's normalization chain is DEFERRED into the next block's
    kt==3 slot; the output projection for a token range fires right
    after its pair-1 normalization, spreading uniformly over the phase.
  - The last block's output projection drains through the freed score
    psum slots, split across the Scalar and Vector engines.

On-chip layouts (per core):
  qt/kt: (128 feat-part, pair, 2048 tok)  transposed proj outputs; the
         128 partitions hold two heads (64+64) per pair index.
  v:     (128 tok-part, 16 tok-tiles, 4*65): per head 64 dims plus a
         "ones" column produced by an augmented V projection (extra
         output feature with zero weights and bias 1.0); P @ V_aug then
         also yields the softmax denominator row for free.
  scores are computed transposed (key-pos on partitions, query on free)
  so exp runs on ACT along the free dim and P tiles feed P@V directly as
  the moving operand; no transposes anywhere in the pipeline.
"""

import numpy as np

B, S, E, H = 2, 2048, 1024, 16
D = 64
NCORES = 8
FPC = 256  # features (head dims) per core = 4 heads
VW = 4 * 65  # V-projection output width incl. ones columns

MODE = "bf16"

_PROGRAMS = {}
LAST_RESULT = None
TRACE = False
TRACE_DIR = None


def _build(mode):
    import concourse.tile as tile
    from concourse import bacc, mybir

    f32 = mybir.dt.float32
    DT = mybir.dt.bfloat16
    NW = 512
    NNB = S // NW  # 4 query blocks per pair

    nc = bacc.Bacc("TRN2", target_bir_lowering=False, debug=False,
                   num_devices=NCORES)

    xq_ap = nc.dram_tensor("xq", [E, S], DT, kind="ExternalInput").ap()
    xk_ap = nc.dram_tensor("xk", [E, S], DT, kind="ExternalInput").ap()
    xv_ap = nc.dram_tensor("xv", [E, S], DT, kind="ExternalInput").ap()
    wq_ap = nc.dram_tensor("wq", [128, 8, FPC], DT, kind="ExternalInput").ap()
    wk_ap = nc.dram_tensor("wk", [128, 8, FPC], DT, kind="ExternalInput").ap()
    wv_ap = nc.dram_tensor("wv", [128, 8, VW], DT, kind="ExternalInput").ap()
    wo_ap = nc.dram_tensor("wo", [128, 2, E], DT, kind="ExternalInput").ap()
    bqk_ap = nc.dram_tensor("bqk", [128, 4], f32, kind="ExternalInput").ap()
    bv_ap = nc.dram_tensor("bv", [1, VW], DT, kind="ExternalInput").ap()
    ones_ap = nc.dram_tensor("ones", [1, 128], DT, kind="ExternalInput").ap()
    y_ap = nc.dram_tensor("y", [S, E], f32, kind="ExternalOutput").ap()

    Exp = mybir.ActivationFunctionType.Exp

    with tile.TileContext(nc) as tc:
        with tc.tile_pool(name="persist", bufs=1) as persist:
            wq_sb = persist.tile([128, 8, FPC], DT, name="wq_sb")
            wk_sb = persist.tile([128, 8, FPC], DT, name="wk_sb")
            wv_sb = persist.tile([128, 8, VW], DT, name="wv_sb")
            wo_sb = persist.tile([128, 2, E], DT, name="wo_sb")
            bqk_sb = persist.tile([128, 4], f32, name="bqk_sb")
            bv_sb = persist.tile([1, VW], DT, name="bv_sb")
            ones_sb = persist.tile([1, 128], DT, name="ones_sb")
            qt_sb = persist.tile([128, 2, S], DT, name="qt_sb")
            kt_sb = persist.tile([128, 2, S], DT, name="kt_sb")
            v_sb = persist.tile([128, 16, VW], DT, name="v_sb")
            at_sb = persist.tile([128, 2, S], DT, name="at_sb")
            xq_res = persist.tile([128, 8, S], DT, name="xq_res")
            xk_res = persist.tile([128, 8, S], DT, name="xk_res")
            xv_res = persist.tile([128, 8, S], DT, name="xv_res")
            ones32 = ones_sb[:, 0:64]

            # weights/consts on the gpsimd DGE queue, q first (needed first)
            nc.gpsimd.dma_start(wq_sb, wq_ap)
            nc.gpsimd.dma_start(bqk_sb, bqk_ap)
            nc.gpsimd.dma_start(wk_sb, wk_ap)
            nc.gpsimd.dma_start(wv_sb, wv_ap)
            nc.gpsimd.dma_start(bv_sb, bv_ap)
            nc.gpsimd.dma_start(ones_sb, ones_ap)
            nc.gpsimd.dma_start(wo_sb, wo_ap)
            # x on TWO queues, alternating tiles, priority xq > xv > xk:
            # attention needs qt+kt+v ready; K-proj is the cheapest PE
            # phase so it tracks the LAST-arriving tensor, while the
            # ldweights-heavy V-projection overlaps xk's DMA window.
            # (3-way striping across gpsimd measured WORSE: its queue is
            # busy with weights and the late tiles stall the f-rounds.)
            for xres, xap in ((xq_res, xq_ap), (xv_res, xv_ap),
                              (xk_res, xk_ap)):
                for f in range(8):
                    eng = nc.sync if f % 2 == 0 else nc.scalar
                    eng.dma_start(xres[:, f, :],
                                  xap[f * 128:(f + 1) * 128, :])

            def emit_qk_proj(pool, w_sb, x_res, out_sb, bcol):
                """f-inner projection, both pairs: 8 psum accumulators
                track the x feature tiles as they arrive."""
                pj = {}
                for p in range(2):
                    for nb in range(NNB):
                        pj[(p, nb)] = pool.tile(
                            [128, NW], f32, tag="proj", bufs=8,
                            name=f"pj_{bcol}_{p}_{nb}")
                for f in range(8):
                    for p in range(2):
                        for nb in range(NNB):
                            nc.tensor.matmul(
                                pj[(p, nb)],
                                w_sb[:, f, p * 128:(p + 1) * 128],
                                x_res[:, f, nb * NW:(nb + 1) * NW],
                                start=(f == 0), stop=(f == 7))
                for p in range(2):
                    for nb in range(NNB):
                        nc.vector.tensor_scalar_add(
                            out_sb[:, p, nb * NW:(nb + 1) * NW], pj[(p, nb)],
                            bqk_sb[:, bcol + p:bcol + p + 1])

            def emit_v_half(pool, half, ntiles=8):
                """V projection for 8 token tiles, f-inner (tracks xv)."""
                psv = [pool.tile([128, VW], f32, tag="proj", bufs=8,
                                 name=f"pjv_{half}_{i}")
                       for i in range(ntiles)]
                for i in range(ntiles):
                    nc.tensor.matmul(psv[i], ones_sb, bv_sb,
                                     start=True, stop=False)
                for f in range(8):
                    for i in range(ntiles):
                        tt = half * 8 + i
                        nc.tensor.matmul(
                            psv[i],
                            xv_res[:, f, tt * 128:(tt + 1) * 128],
                            wv_sb[:, f, :],
                            start=False, stop=(f == 7))
                for i in range(ntiles):
                    nc.vector.tensor_copy(v_sb[:, half * 8 + i, :], psv[i])

            # ---- lead-in: Q-proj then V-proj (token tiles 0..13) ----
            # K-proj is NOT here: it would serialize behind the V passes
            # in the in-order PE queue. It runs as per-key-block chunks
            # interleaved into the attention stream below, racing the
            # score consumption (xk is resident by then). The last two V
            # tiles also move into the stream so the first scores start
            # sooner.
            with tc.tile_pool(name="lead", bufs=1, space="PSUM") as lead:
                emit_qk_proj(lead, wq_sb, xq_res, qt_sb, 0)
                emit_v_half(lead, 0)
                emit_v_half(lead, 1, ntiles=6)

            # ---- attention: flat unit stream, 3-deep PV skew ----
            # Units are (qb, p, kt); the PE emits scores(u_i) then
            # PV(u_{i-3}) so exp latency on the Scalar engine never blocks
            # the in-order PE queue — including across block boundaries.
            # Each block's normalization chain is deferred into the next
            # block's kt==3 slot (just before that block's pv(0), whose
            # pvt slot rotation waits on this at-mul).
            # psum pool order matters: banks are assigned in order and the
            # released lead-pool's banks free progressively (v-pass2 drains
            # tt8..tt15); misc (first K-proj chunk) and the score slots
            # want the earliest-freed banks.
            with tc.tile_pool(name="pt", bufs=14) as ptpool, \
                 tc.tile_pool(name="sm", bufs=2) as smpool, \
                 tc.tile_pool(name="ysb", bufs=2) as ypool, \
                 tc.tile_pool(name="miscps", bufs=2, space="PSUM") as miscps, \
                 tc.tile_pool(name="scps", bufs=2, space="PSUM") as scps, \
                 tc.tile_pool(name="pvps", bufs=2, space="PSUM") as pvps:

                pvt_of = {}  # (qb, p) -> [h0_tile, h1_tile]

                def emit_yproj_block(qb, last):
                    for mt in range(4 * qb, 4 * qb + 4):
                        yo = ypool.tile([128, E], f32, tag="yo",
                                        name=f"yo_{mt}")
                        for nb in range(2):
                            # the last block's yp tiles use the score psum
                            # slots (free once the exps are done) so the
                            # tail pipelines on 2 slots instead of 1
                            yp = (scps if last else miscps).tile(
                                [128, NW], f32,
                                tag="sc" if last else "misc",
                                name=f"yp_{mt}_{nb}")
                            for p2 in range(2):
                                nc.tensor.matmul(
                                    yp,
                                    at_sb[:, p2, mt * 128:(mt + 1) * 128],
                                    wo_sb[:, p2, nb * NW:(nb + 1) * NW],
                                    start=(p2 == 0), stop=(p2 == 1))
                            # gpsimd cannot read PSUM; drain via DVE, and
                            # split across the (by then idle) Scalar engine
                            # at the tail so the two streams drain in
                            # parallel
                            if last and (mt + nb) % 2 == 0:
                                nc.scalar.copy(
                                    yo[:, nb * NW:(nb + 1) * NW], yp)
                            else:
                                nc.vector.tensor_copy(
                                    yo[:, nb * NW:(nb + 1) * NW], yp)
                        eng = nc.gpsimd if mt % 2 else nc.sync
                        eng.dma_start(y_ap[mt * 128:(mt + 1) * 128, :], yo)

                def emit_norm(qb, p, last=False):
                    qsl = slice(qb * NW, (qb + 1) * NW)
                    pvt = pvt_of.pop((qb, p))
                    for hh in range(2):
                        denr = smpool.tile([1, NW], DT, tag="denr",
                                           name=f"dn_{p}_{qb}_{hh}")
                        nc.vector.tensor_copy(denr, pvt[hh][64:65, :])
                        rb = miscps.tile([64, NW], f32, tag="misc",
                                         name=f"rb_{p}_{qb}_{hh}")
                        nc.tensor.matmul(rb, ones32, denr,
                                         start=True, stop=True)
                        rbs = smpool.tile([64, NW], f32, tag="rbs",
                                          name=f"rbs_{p}_{qb}_{hh}")
                        nc.vector.reciprocal_approx_fast(rbs, rb)
                        nc.vector.tensor_mul(
                            at_sb[64 * hh:64 * hh + 64, p, qsl],
                            pvt[hh][0:64, :], rbs)
                    if p == 1:
                        emit_yproj_block(qb, last)

                def emit_kt_chunk(p, nb):
                    """K projection for one key block (8 f-matmuls on a
                    rotating misc psum slot, DVE bias-add drain)."""
                    pj = miscps.tile([128, NW], f32, tag="misc",
                                     name=f"pk_{p}_{nb}")
                    for f in range(8):
                        nc.tensor.matmul(
                            pj,
                            wk_sb[:, f, p * 128:(p + 1) * 128],
                            xk_res[:, f, nb * NW:(nb + 1) * NW],
                            start=(f == 0), stop=(f == 7))
                    nc.vector.tensor_scalar_add(
                        kt_sb[:, p, nb * NW:(nb + 1) * NW], pj,
                        bqk_sb[:, 2 + p:3 + p])

                def emit_v_chunk(tt):
                    """V projection for one token tile on the misc slot."""
                    psv = miscps.tile([128, VW], f32, tag="misc",
                                      name=f"pjv_s_{tt}")
                    nc.tensor.matmul(psv, ones_sb, bv_sb,
                                     start=True, stop=False)
                    for f in range(8):
                        nc.tensor.matmul(
                            psv,
                            xv_res[:, f, tt * 128:(tt + 1) * 128],
                            wv_sb[:, f, :],
                            start=False, stop=(f == 7))
                    nc.vector.tensor_copy(v_sb[:, tt, :], psv)

                units = [(qb, p, kt)
                         for qb in range(NNB) for p in range(2)
                         for kt in range(16)]
                pend = []  # pv emissions in flight (skew 3)
                # K-proj chunk (p, nb) produces key tiles nb*4..nb*4+3 of
                # pair p; chunk i must precede the first unit consuming it
                # (unit kt==4i%16 of pair i//4 in qb0). Emit chunk 0 up
                # front, then one chunk every 4 units.
                emit_kt_chunk(0, 0)
                kt_chunks = [(p, nb) for p in range(2) for nb in range(4)][1:]

                def emit_pv(qb, p, kt, ptt):
                    if kt == 0:
                        pvt_of[(qb, p)] = [
                            pvps.tile([65, NW], f32, tag="pv",
                                      name=f"pv_{p}_{qb}_{hh}")
                            for hh in range(2)]
                    pvt = pvt_of[(qb, p)]
                    for hh in range(2):
                        h = 2 * p + hh
                        nc.tensor.matmul(
                            pvt[hh],
                            v_sb[:, kt, 65 * h:65 * h + 65],
                            ptt[:, NW * hh:NW * hh + NW],
                            start=(kt == 0), stop=(kt == 15))

                for i, (qb, p, kt) in enumerate(units):
                    if i in (1, 2):
                        emit_v_chunk(13 + i)  # V token tiles 14, 15
                    if kt_chunks and i % 4 == 3:
                        emit_kt_chunk(*kt_chunks.pop(0))
                    qsl = slice(qb * NW, (qb + 1) * NW)
                    s_ = scps.tile([128, 2 * NW], f32, tag="sc",
                                   name=f"sc_{p}_{qb}_{kt}")
                    for hh in range(2):
                        nc.tensor.matmul(
                            s_[:, NW * hh:NW * hh + NW],
                            kt_sb[64 * hh:64 * hh + 64, p,
                                  kt * 128:(kt + 1) * 128],
                            qt_sb[64 * hh:64 * hh + 64, p, qsl],
                            start=True, stop=True)
                    ptt = ptpool.tile([128, 2 * NW], DT, tag="pt",
                                      name=f"pt_{p}_{qb}_{kt}")
                    nc.scalar.activation(ptt, s_, Exp, scale=0.125)
                    pend.append((qb, p, kt, ptt))
                    if kt == 3 and (qb, p) != (0, 0):
                        # previous block's pv(15) was emitted one unit ago;
                        # normalize it now, BEFORE this block's pv(0) below
                        # (whose pvt slot rotation waits on this at-mul).
                        pqb, pp = (qb, p - 1) if p == 1 else (qb - 1, 1)
                        emit_norm(pqb, pp)
                    if len(pend) > 3:
                        emit_pv(*pend.pop(0))
                while pend:
                    emit_pv(*pend.pop(0))
                emit_norm(NNB - 1, 1, last=True)

    nc.compile()
    return nc


def _get_program(mode):
    if mode not in _PROGRAMS:
        _PROGRAMS[mode] = _build(mode)
    return _PROGRAMS[mode]


def kernel(q, k, v, mask, Wq, bq, Wk, bk, Wv, bv, Wo, bo):
    global LAST_RESULT
    from concourse.bass_utils import run_bass_kernel_spmd

    mode = MODE
    nc = _get_program(mode)

    import ml_dtypes
    cdt = ml_dtypes.bfloat16

    def prep(a):
        return np.ascontiguousarray(np.asarray(a).astype(cdt))

    q = np.asarray(q); k = np.asarray(k); v = np.asarray(v)
    Wq = np.asarray(Wq); Wk = np.asarray(Wk); Wv = np.asarray(Wv)
    Wo = np.asarray(Wo)
    bq = np.asarray(bq); bk = np.asarray(bk); bv = np.asarray(bv)
    bo = np.asarray(bo)

    xT = [[prep(q[b].T), prep(k[b].T), prep(v[b].T)] for b in range(B)]

    in_maps = []
    for core in range(NCORES):
        b, g = core // 4, core % 4
        r0 = g * FPC

        def wqk_layout(W):
            # lhsT tiles: [part p, ktile, m] = W.T[kt*128+p, m]
            A = W[r0:r0 + FPC, :].T.reshape(8, 128, FPC)
            return prep(A.transpose(1, 0, 2))

        WvT = Wv[r0:r0 + FPC, :].T  # (E, 256)
        Wv_aug = np.zeros((E, VW), np.float32)
        bv_aug = np.zeros((1, VW), np.float32)
        for h in range(4):
            Wv_aug[:, 65 * h:65 * h + 64] = WvT[:, 64 * h:64 * h + 64]
            bv_aug[0, 65 * h:65 * h + 64] = bv[r0 + 64 * h:r0 + 64 * h + 64]
            bv_aug[0, 65 * h + 64] = 1.0
        Wo_l = Wo[:, r0:r0 + FPC].T.reshape(2, 128, E).transpose(1, 0, 2)

        in_maps.append({
            "xq": xT[b][0], "xk": xT[b][1], "xv": xT[b][2],
            "wq": wqk_layout(Wq),
            "wk": wqk_layout(Wk),
            "wv": prep(Wv_aug.reshape(8, 128, VW).transpose(1, 0, 2)),
            "wo": prep(Wo_l),
            "bqk": np.stack([bq[r0:r0 + 128], bq[r0 + 128:r0 + FPC],
                             bk[r0:r0 + 128], bk[r0 + 128:r0 + FPC]],
                            axis=1).astype(np.float32),
            "bv": prep(bv_aug),
            "ones": np.ones((1, 128), cdt),
        })

    kwargs = {}
    if TRACE:
        kwargs = {"trace": True, "tmpdir": TRACE_DIR}
    res = run_bass_kernel_spmd(nc, in_maps, list(range(NCORES)), **kwargs)
    LAST_RESULT = res

    y = np.zeros((B, S, E), np.float32)
    for core in range(NCORES):
        y[core // 4] += res.results[core]["y"]
    y += bo.astype(np.float32)
    return y


# revision 48
# speedup vs baseline: 1.0777x; 1.0307x over previous
"""Multi-head attention (B=2, S=2048, E=1024, H=16) on 8 Trainium2 cores.

Sharding: core c -> (batch b = c//4, head-group g = c%4 of 4 heads).
Each core computes Q/K/V projections for its 4 heads (256 features),
full attention for those heads, and a partial output projection
(256 rows of Wo). Host sums the 4 partials per batch element and adds bo.

Schedule (final): built for engine overlap around three facts measured
from NTFF traces: the in-order PE queue executes exactly in emission
order, the exp stream on the Scalar engine is the steady-state floor
(~1.09us per key-tile unit), and the PE p-state only reaches 2.4GHz
when kept continuously busy.
  - x DMAs ride TWO queues (sync+scalar, alternating feature tiles) in
    priority order xq, xv, xk; weights ride the gpsimd queue. y output
    rides sync/gpsimd (NOT scalar: the Scalar engine is the exp
    bottleneck and DMA triggers would steal ~0.7us each from it).
  - Lead-in: Q-proj tracks xq arrival f-inner on 8 psum accumulators,
    then V-proj (token tiles 0-13) tracks xv. K-proj and the last two V
    tiles run as chunks interleaved INTO the attention stream (racing
    the score consumption) so nothing serializes behind the
    last-arriving tensor.
  - Attention is a flat software-pipelined unit stream (qb-major,
    3-deep PV skew): the PE emits scores(u_i) then PV(u_{i-3}), so exp
    latency never blocks the in-order PE queue — including across
    block boundaries.
  - Each block's normalization chain is DEFERRED into the next block's
    kt==3 slot; the output projection for a token range fires right
    after its pair-1 normalization, spreading uniformly over the phase.
  - The last block's output projection drains through the freed score
    psum slots, split across the Scalar and Vector engines.

On-chip layouts (per core):
  qt/kt: (128 feat-part, pair, 2048 tok)  transposed proj outputs; the
         128 partitions hold two heads (64+64) per pair index.
  v:     (128 tok-part, 16 tok-tiles, 4*65): per head 64 dims plus a
         "ones" column produced by an augmented V projection (extra
         output feature with zero weights and bias 1.0); P @ V_aug then
         also yields the softmax denominator row for free.
  scores are computed transposed (key-pos on partitions, query on free)
  so exp runs on ACT along the free dim and P tiles feed P@V directly as
  the moving operand; no transposes anywhere in the pipeline.
"""

import numpy as np

B, S, E, H = 2, 2048, 1024, 16
D = 64
NCORES = 8
FPC = 256  # features (head dims) per core = 4 heads
VW = 4 * 65  # V-projection output width incl. ones columns

MODE = "bf16"

_PROGRAMS = {}
LAST_RESULT = None
TRACE = False
TRACE_DIR = None


def _build(mode):
    import concourse.tile as tile
    from concourse import bacc, mybir

    f32 = mybir.dt.float32
    DT = mybir.dt.bfloat16
    NW = 512
    NNB = S // NW  # 4 query blocks per pair

    nc = bacc.Bacc("TRN2", target_bir_lowering=False, debug=False,
                   num_devices=NCORES)

    xq_ap = nc.dram_tensor("xq", [E, S], DT, kind="ExternalInput").ap()
    xk_ap = nc.dram_tensor("xk", [E, S], DT, kind="ExternalInput").ap()
    xv_ap = nc.dram_tensor("xv", [E, S], DT, kind="ExternalInput").ap()
    wq_ap = nc.dram_tensor("wq", [128, 8, FPC], DT, kind="ExternalInput").ap()
    wk_ap = nc.dram_tensor("wk", [128, 8, FPC], DT, kind="ExternalInput").ap()
    wv_ap = nc.dram_tensor("wv", [128, 8, VW], DT, kind="ExternalInput").ap()
    wo_ap = nc.dram_tensor("wo", [128, 2, E], DT, kind="ExternalInput").ap()
    bqk_ap = nc.dram_tensor("bqk", [128, 4], f32, kind="ExternalInput").ap()
    bv_ap = nc.dram_tensor("bv", [1, VW], DT, kind="ExternalInput").ap()
    ones_ap = nc.dram_tensor("ones", [1, 128], DT, kind="ExternalInput").ap()
    y_ap = nc.dram_tensor("y", [S, E], f32, kind="ExternalOutput").ap()

    Exp = mybir.ActivationFunctionType.Exp

    with tile.TileContext(nc) as tc:
        with tc.tile_pool(name="persist", bufs=1) as persist:
            wq_sb = persist.tile([128, 8, FPC], DT, name="wq_sb")
            wk_sb = persist.tile([128, 8, FPC], DT, name="wk_sb")
            wv_sb = persist.tile([128, 8, VW], DT, name="wv_sb")
            wo_sb = persist.tile([128, 2, E], DT, name="wo_sb")
            bqk_sb = persist.tile([128, 4], f32, name="bqk_sb")
            bv_sb = persist.tile([1, VW], DT, name="bv_sb")
            ones_sb = persist.tile([1, 128], DT, name="ones_sb")
            qt_sb = persist.tile([128, 2, S], DT, name="qt_sb")
            kt_sb = persist.tile([128, 2, S], DT, name="kt_sb")
            v_sb = persist.tile([128, 16, VW], DT, name="v_sb")
            at_sb = persist.tile([128, 2, S], DT, name="at_sb")
            xq_res = persist.tile([128, 8, S], DT, name="xq_res")
            xk_res = persist.tile([128, 8, S], DT, name="xk_res")
            xv_res = persist.tile([128, 8, S], DT, name="xv_res")
            ones32 = ones_sb[:, 0:64]

            # weights/consts on the gpsimd DGE queue, q first (needed first)
            nc.gpsimd.dma_start(wq_sb, wq_ap)
            nc.gpsimd.dma_start(bqk_sb, bqk_ap)
            nc.gpsimd.dma_start(wk_sb, wk_ap)
            nc.gpsimd.dma_start(wv_sb, wv_ap)
            nc.gpsimd.dma_start(bv_sb, bv_ap)
            nc.gpsimd.dma_start(ones_sb, ones_ap)
            nc.gpsimd.dma_start(wo_sb, wo_ap)
            # x on TWO queues, alternating tiles, priority xq > xv > xk:
            # attention needs qt+kt+v ready; K-proj is the cheapest PE
            # phase so it tracks the LAST-arriving tensor, while the
            # ldweights-heavy V-projection overlaps xk's DMA window.
            # (3-way striping across gpsimd measured WORSE: its queue is
            # busy with weights and the late tiles stall the f-rounds.)
            for xres, xap in ((xq_res, xq_ap), (xv_res, xv_ap),
                              (xk_res, xk_ap)):
                for f in range(8):
                    eng = nc.sync if f % 2 == 0 else nc.scalar
                    eng.dma_start(xres[:, f, :],
                                  xap[f * 128:(f + 1) * 128, :])

            def emit_qk_proj(pool, w_sb, x_res, out_sb, bcol):
                """f-inner projection, both pairs: 8 psum accumulators
                track the x feature tiles as they arrive."""
                pj = {}
                for p in range(2):
                    for nb in range(NNB):
                        pj[(p, nb)] = pool.tile(
                            [128, NW], f32, tag="proj", bufs=8,
                            name=f"pj_{bcol}_{p}_{nb}")
                for f in range(8):
                    for p in range(2):
                        for nb in range(NNB):
                            nc.tensor.matmul(
                                pj[(p, nb)],
                                w_sb[:, f, p * 128:(p + 1) * 128],
                                x_res[:, f, nb * NW:(nb + 1) * NW],
                                start=(f == 0), stop=(f == 7))
                for p in range(2):
                    for nb in range(NNB):
                        nc.vector.tensor_scalar_add(
                            out_sb[:, p, nb * NW:(nb + 1) * NW], pj[(p, nb)],
                            bqk_sb[:, bcol + p:bcol + p + 1])

            def emit_v_half(pool, half, ntiles=8):
                """V projection for 8 token tiles, f-inner (tracks xv)."""
                psv = [pool.tile([128, VW], f32, tag="proj", bufs=8,
                                 name=f"pjv_{half}_{i}")
                       for i in range(ntiles)]
                for i in range(ntiles):
                    nc.tensor.matmul(psv[i], ones_sb, bv_sb,
                                     start=True, stop=False)
                for f in range(8):
                    for i in range(ntiles):
                        tt = half * 8 + i
                        nc.tensor.matmul(
                            psv[i],
                            xv_res[:, f, tt * 128:(tt + 1) * 128],
                            wv_sb[:, f, :],
                            start=False, stop=(f == 7))
                for i in range(ntiles):
                    nc.vector.tensor_copy(v_sb[:, half * 8 + i, :], psv[i])

            # ---- lead-in: Q-proj then V-proj (token tiles 0..13) ----
            # K-proj is NOT here: it would serialize behind the V passes
            # in the in-order PE queue. It runs as per-key-block chunks
            # interleaved into the attention stream below, racing the
            # score consumption (xk is resident by then). The last two V
            # tiles also move into the stream so the first scores start
            # sooner.
            with tc.tile_pool(name="lead", bufs=1, space="PSUM") as lead:
                emit_qk_proj(lead, wq_sb, xq_res, qt_sb, 0)
                emit_v_half(lead, 0)
                emit_v_half(lead, 1, ntiles=6)

            # ---- attention: flat unit stream, 3-deep PV skew ----
            # Units are (qb, p, kt); the PE emits scores(u_i) then
            # PV(u_{i-3}) so exp latency on the Scalar engine never blocks
            # the in-order PE queue — including across block boundaries.
            # Each block's normalization chain is deferred into the next
            # block's kt==3 slot (just before that block's pv(0), whose
            # pvt slot rotation waits on this at-mul).
            # psum pool order matters: banks are assigned in order and the
            # released lead-pool's banks free progressively (v-pass2 drains
            # tt8..tt15); misc (first K-proj chunk) and the score slots
            # want the earliest-freed banks.
            with tc.tile_pool(name="pt", bufs=14) as ptpool, \
                 tc.tile_pool(name="sm", bufs=2) as smpool, \
                 tc.tile_pool(name="ysb", bufs=2) as ypool, \
                 tc.tile_pool(name="miscps", bufs=1, space="PSUM") as miscps, \
                 tc.tile_pool(name="scps", bufs=2, space="PSUM") as scps, \
                 tc.tile_pool(name="pvps", bufs=3, space="PSUM") as pvps:

                pvt_of = {}  # (qb, p) -> [h0_tile, h1_tile]

                def emit_yproj_block(qb, last):
                    for mt in range(4 * qb, 4 * qb + 4):
                        yo = ypool.tile([128, E], f32, tag="yo",
                                        name=f"yo_{mt}")
                        for nb in range(2):
                            # the last block's yp tiles use the score psum
                            # slots (free once the exps are done) so the
                            # tail pipelines on 2 slots instead of 1
                            yp = (scps if last else miscps).tile(
                                [128, NW], f32,
                                tag="sc" if last else "misc",
                                name=f"yp_{mt}_{nb}")
                            for p2 in range(2):
                                nc.tensor.matmul(
                                    yp,
                                    at_sb[:, p2, mt * 128:(mt + 1) * 128],
                                    wo_sb[:, p2, nb * NW:(nb + 1) * NW],
                                    start=(p2 == 0), stop=(p2 == 1))
                            # gpsimd cannot read PSUM; drain via DVE, and
                            # split across the (by then idle) Scalar engine
                            # at the tail so the two streams drain in
                            # parallel
                            if last and (mt + nb) % 2 == 0:
                                nc.scalar.copy(
                                    yo[:, nb * NW:(nb + 1) * NW], yp)
                            else:
                                nc.vector.tensor_copy(
                                    yo[:, nb * NW:(nb + 1) * NW], yp)
                        eng = nc.gpsimd if mt % 2 else nc.sync
                        eng.dma_start(y_ap[mt * 128:(mt + 1) * 128, :], yo)

                def emit_norm(qb, p, last=False):
                    qsl = slice(qb * NW, (qb + 1) * NW)
                    pvt = pvt_of.pop((qb, p))
                    for hh in range(2):
                        denr = smpool.tile([1, NW], DT, tag="denr",
                                           name=f"dn_{p}_{qb}_{hh}")
                        nc.vector.tensor_copy(denr, pvt[hh][64:65, :])
                        rb = miscps.tile([64, NW], f32, tag="misc",
                                         name=f"rb_{p}_{qb}_{hh}")
                        nc.tensor.matmul(rb, ones32, denr,
                                         start=True, stop=True)
                        rbs = smpool.tile([64, NW], f32, tag="rbs",
                                          name=f"rbs_{p}_{qb}_{hh}")
                        nc.vector.reciprocal_approx_fast(rbs, rb)
                        nc.vector.tensor_mul(
                            at_sb[64 * hh:64 * hh + 64, p, qsl],
                            pvt[hh][0:64, :], rbs)
                    if p == 1:
                        emit_yproj_block(qb, last)

                def emit_kt_chunk(p, nb):
                    """K projection for one key block (8 f-matmuls on a
                    rotating misc psum slot, DVE bias-add drain)."""
                    pj = miscps.tile([128, NW], f32, tag="misc",
                                     name=f"pk_{p}_{nb}")
                    for f in range(8):
                        nc.tensor.matmul(
                            pj,
                            wk_sb[:, f, p * 128:(p + 1) * 128],
                            xk_res[:, f, nb * NW:(nb + 1) * NW],
                            start=(f == 0), stop=(f == 7))
                    nc.vector.tensor_scalar_add(
                        kt_sb[:, p, nb * NW:(nb + 1) * NW], pj,
                        bqk_sb[:, 2 + p:3 + p])

                def emit_v_chunk(tt):
                    """V projection for one token tile on the misc slot."""
                    psv = miscps.tile([128, VW], f32, tag="misc",
                                      name=f"pjv_s_{tt}")
                    nc.tensor.matmul(psv, ones_sb, bv_sb,
                                     start=True, stop=False)
                    for f in range(8):
                        nc.tensor.matmul(
                            psv,
                            xv_res[:, f, tt * 128:(tt + 1) * 128],
                            wv_sb[:, f, :],
                            start=False, stop=(f == 7))
                    nc.vector.tensor_copy(v_sb[:, tt, :], psv)

                units = [(qb, p, kt)
                         for qb in range(NNB) for p in range(2)
                         for kt in range(16)]
                pend = []  # pv emissions in flight (skew 3)
                # K-proj chunk (p, nb) produces key tiles nb*4..nb*4+3 of
                # pair p; chunk i must precede the first unit consuming it
                # (unit kt==4i%16 of pair i//4 in qb0). Emit chunk 0 up
                # front, then one chunk every 4 units.
                emit_kt_chunk(0, 0)
                kt_chunks = [(p, nb) for p in range(2) for nb in range(4)][1:]

                def emit_pv(qb, p, kt, ptt):
                    if kt == 0:
                        pvt_of[(qb, p)] = [
                            pvps.tile([65, NW], f32, tag="pv",
                                      name=f"pv_{p}_{qb}_{hh}")
                            for hh in range(2)]
                    pvt = pvt_of[(qb, p)]
                    for hh in range(2):
                        h = 2 * p + hh
                        nc.tensor.matmul(
                            pvt[hh],
                            v_sb[:, kt, 65 * h:65 * h + 65],
                            ptt[:, NW * hh:NW * hh + NW],
                            start=(kt == 0), stop=(kt == 15))

                for i, (qb, p, kt) in enumerate(units):
                    if i in (1, 2):
                        emit_v_chunk(13 + i)  # V token tiles 14, 15
                    if kt_chunks and i % 4 == 3:
                        emit_kt_chunk(*kt_chunks.pop(0))
                    qsl = slice(qb * NW, (qb + 1) * NW)
                    s_ = scps.tile([128, 2 * NW], f32, tag="sc",
                                   name=f"sc_{p}_{qb}_{kt}")
                    for hh in range(2):
                        nc.tensor.matmul(
                            s_[:, NW * hh:NW * hh + NW],
                            kt_sb[64 * hh:64 * hh + 64, p,
                                  kt * 128:(kt + 1) * 128],
                            qt_sb[64 * hh:64 * hh + 64, p, qsl],
                            start=True, stop=True)
                    ptt = ptpool.tile([128, 2 * NW], DT, tag="pt",
                                      name=f"pt_{p}_{qb}_{kt}")
                    nc.scalar.activation(ptt, s_, Exp, scale=0.125)
                    pend.append((qb, p, kt, ptt))
                    if kt == 3 and (qb, p) != (0, 0):
                        # previous block's pv(15) was emitted one unit ago;
                        # normalize it now, BEFORE this block's pv(0) below
                        # (whose pvt slot rotation waits on this at-mul).
                        pqb, pp = (qb, p - 1) if p == 1 else (qb - 1, 1)
                        emit_norm(pqb, pp)
                    if len(pend) > 3:
                        emit_pv(*pend.pop(0))
                while pend:
                    emit_pv(*pend.pop(0))
                emit_norm(NNB - 1, 1, last=True)

    nc.compile()
    return nc


def _get_program(mode):
    if mode not in _PROGRAMS:
        _PROGRAMS[mode] = _build(mode)
    return _PROGRAMS[mode]


def kernel(q, k, v, mask, Wq, bq, Wk, bk, Wv, bv, Wo, bo):
    global LAST_RESULT
    from concourse.bass_utils import run_bass_kernel_spmd

    mode = MODE
    nc = _get_program(mode)

    import ml_dtypes
    cdt = ml_dtypes.bfloat16

    def prep(a):
        return np.ascontiguousarray(np.asarray(a).astype(cdt))

    q = np.asarray(q); k = np.asarray(k); v = np.asarray(v)
    Wq = np.asarray(Wq); Wk = np.asarray(Wk); Wv = np.asarray(Wv)
    Wo = np.asarray(Wo)
    bq = np.asarray(bq); bk = np.asarray(bk); bv = np.asarray(bv)
    bo = np.asarray(bo)

    xT = [[prep(q[b].T), prep(k[b].T), prep(v[b].T)] for b in range(B)]

    in_maps = []
    for core in range(NCORES):
        b, g = core // 4, core % 4
        r0 = g * FPC

        def wqk_layout(W):
            # lhsT tiles: [part p, ktile, m] = W.T[kt*128+p, m]
            A = W[r0:r0 + FPC, :].T.reshape(8, 128, FPC)
            return prep(A.transpose(1, 0, 2))

        WvT = Wv[r0:r0 + FPC, :].T  # (E, 256)
        Wv_aug = np.zeros((E, VW), np.float32)
        bv_aug = np.zeros((1, VW), np.float32)
        for h in range(4):
            Wv_aug[:, 65 * h:65 * h + 64] = WvT[:, 64 * h:64 * h + 64]
            bv_aug[0, 65 * h:65 * h + 64] = bv[r0 + 64 * h:r0 + 64 * h + 64]
            bv_aug[0, 65 * h + 64] = 1.0
        Wo_l = Wo[:, r0:r0 + FPC].T.reshape(2, 128, E).transpose(1, 0, 2)

        in_maps.append({
            "xq": xT[b][0], "xk": xT[b][1], "xv": xT[b][2],
            "wq": wqk_layout(Wq),
            "wk": wqk_layout(Wk),
            "wv": prep(Wv_aug.reshape(8, 128, VW).transpose(1, 0, 2)),
            "wo": prep(Wo_l),
            "bqk": np.stack([bq[r0:r0 + 128], bq[r0 + 128:r0 + FPC],
                             bk[r0:r0 + 128], bk[r0 + 128:r0 + FPC]],
                            axis=1).astype(np.float32),
            "bv": prep(bv_aug),
            "ones": np.ones((1, 128), cdt),
        })

    kwargs = {}
    if TRACE:
        kwargs = {"trace": True, "tmpdir": TRACE_DIR}
    res = run_bass_kernel_spmd(nc, in_maps, list(range(NCORES)), **kwargs)
    LAST_RESULT = res

    y = np.zeros((B, S, E), np.float32)
    for core in range(NCORES):
        y[core // 4] += res.results[core]["y"]
    y += bo.astype(np.float32)
    return y
